# revision 2
# baseline (speedup 1.0000x reference)
"""CoAttention ImageDNS kernel for Trainium2 (8 NeuronCores, Bass/Tile).

Math: the reference computes two additive-attention blocks. In both, the
softmax'd score is  score[b, q, k] = f(q-side)[b, q] + g(k-side)[b, k] + c,
and softmax over k is invariant to the q-dependent (and constant) terms, so
the attention weights are independent of the query index:

  visual_att[b, s, :]  = softmax_r( wB . tanh(W_i1 @ img[b, r]) )
  textual_att[b, i, :] = softmax_j( wD . tanh(W_d2 @ dns[b, j]) )

Hence both outputs are per-batch rank-1 broadcasts:

  att_img_features[b, s, :] = visual_att[b]  @ img[b]   (same for all s)
  att_dns_features[b, i, :] = textual_att[b] @ dns[b]   (same for all i)

W_d1/b_d1/w_att1[:H]/b_att1/W_i2/b_i2/w_att2[:H]/b_att2 cancel entirely.

Sharding: pure data-parallel over batch, 4 batches per core, no collectives.
Matmul operands are fp16 (verified end-to-end rel err ~3e-4 vs fp32 ref);
accumulation is fp32 in PSUM, softmax/normalization in fp32.
"""

import sys
import numpy as np

for _p in ("/opt/trn_rl_repo", "/root/.axon_site/_ro/trn_rl_repo"):
    if _p not in sys.path:
        sys.path.append(_p)

B, S, R, H = 32, 512, 196, 1024
NCORES = 8
BLOC = B // NCORES          # batches per core
OC = 512                    # output-chunk (one fp32 PSUM bank)
HC = H // 128               # contraction chunks

_CACHE = {}


def _row_chunks(n):
    out, o = [], 0
    while o < n:
        out.append((o, min(128, n - o)))
        o += 128
    return out


def build_nc():
    from concourse import bacc, mybir
    from concourse import tile

    f32, f16 = mybir.dt.float32, mybir.dt.float16
    Act = mybir.ActivationFunctionType
    Alu = mybir.AluOpType

    nc = bacc.Bacc("TRN2", target_bir_lowering=False, debug=False)

    xt_dns = nc.dram_tensor("xt_dns", [BLOC, HC, 128, S], f16, kind="ExternalInput")
    xn_dns = nc.dram_tensor("xn_dns", [BLOC, S, H], f16, kind="ExternalInput")
    xt_img = nc.dram_tensor("xt_img", [BLOC, HC, 128, R], f16, kind="ExternalInput")
    xn_img = nc.dram_tensor("xn_img", [BLOC, R, H], f16, kind="ExternalInput")
    wt_i1 = nc.dram_tensor("wt_i1", [HC, 128, H], f16, kind="ExternalInput")
    wt_d2 = nc.dram_tensor("wt_d2", [HC, 128, H], f16, kind="ExternalInput")
    wrow_b = nc.dram_tensor("wrow_b", [128, H], f32, kind="ExternalInput")
    wrow_d = nc.dram_tensor("wrow_d", [128, H], f32, kind="ExternalInput")
    out_dns = nc.dram_tensor("out_dns", [BLOC, S, H], f32, kind="ExternalOutput")
    out_img = nc.dram_tensor("out_img", [BLOC, S, H], f32, kind="ExternalOutput")

    with tile.TileContext(nc) as tc:
        with (
            tc.tile_pool(name="const", bufs=1) as cpool,
            tc.tile_pool(name="xts", bufs=2) as xtpool,
            tc.tile_pool(name="xns", bufs=2) as xnpool,
            tc.tile_pool(name="work", bufs=3) as wpool,
            tc.tile_pool(name="small", bufs=12) as spool,
            tc.tile_pool(name="outs", bufs=2) as opool,
            tc.tile_pool(name="pp", bufs=3, space="PSUM") as ppool,
            tc.tile_pool(name="ps", bufs=4, space="PSUM") as pstat,
        ):
            wt_sb = {}
            for nm, dram in (("i1", wt_i1), ("d2", wt_d2)):
                w = cpool.tile([128, HC * H], f16, name=f"wt_{nm}_sb")
                for hc in range(HC):
                    nc.sync.dma_start(out=w[:, hc * H:(hc + 1) * H], in_=dram[hc])
                wt_sb[nm] = w
            wrow_sb = {}
            for nm, dram in (("b", wrow_b), ("d", wrow_d)):
                w = cpool.tile([128, H], f32, name=f"wrow_{nm}_sb")
                nc.sync.dma_start(out=w[:, :], in_=dram[:, :])
                wrow_sb[nm] = w
            ones_col = cpool.tile([128, 1], f16, name="ones_col")
            nc.vector.memset(ones_col[:, :], 1.0)
            ones_row = cpool.tile([1, 128], f32, name="ones_row")
            nc.vector.memset(ones_row[:, :], 1.0)

            for b in range(BLOC):
                for side in ("img", "dns"):
                    n_rows = R if side == "img" else S
                    xt_d = xt_img if side == "img" else xt_dns
                    xn_d = xn_img if side == "img" else xn_dns
                    wt = wt_sb["i1" if side == "img" else "d2"]
                    wr = wrow_sb["b" if side == "img" else "d"]
                    out_d = out_img if side == "img" else out_dns
                    rcs = _row_chunks(n_rows)

                    # -- loads --
                    xt_t = xtpool.tile([128, HC * n_rows], f16,
                                       name=f"xt_{side}_{b}", tag=f"xt_{side}")
                    for hc in range(HC):
                        nc.sync.dma_start(
                            out=xt_t[:, hc * n_rows:(hc + 1) * n_rows],
                            in_=xt_d[b, hc])
                    xn_ts = []
                    for ci, (r0, rk) in enumerate(rcs):
                        t = xnpool.tile([128, H], f16, name=f"xn_{side}_{ci}_{b}",
                                        tag=f"xn_{side}_{ci}")
                        nc.sync.dma_start(out=t[0:rk, :], in_=xn_d[b, r0:r0 + rk, :])
                        xn_ts.append(t)

                    # -- projection, tanh, weighted o-reduction, exp --
                    acols = []
                    s_ps = pstat.tile([1, 1], f32, name=f"s_{side}_{b}", tag="stat")
                    for ci, (r0, rk) in enumerate(rcs):
                        ps = [ppool.tile([128, OC], f32, name=f"proj{oc}_{side}_{ci}_{b}",
                                         tag="pp") for oc in range(2)]
                        for hc in range(HC):
                            lhs = xt_t[:, hc * n_rows + r0: hc * n_rows + r0 + rk]
                            for oc in range(2):
                                nc.tensor.matmul(
                                    ps[oc][0:rk, :],
                                    lhsT=lhs,
                                    rhs=wt[:, hc * H + oc * OC: hc * H + (oc + 1) * OC],
                                    start=(hc == 0), stop=(hc == HC - 1))
                        th = wpool.tile([128, H], f32, name=f"th_{side}_{ci}_{b}", tag="th")
                        for oc in range(2):
                            nc.scalar.activation(th[0:rk, oc * OC:(oc + 1) * OC],
                                                 ps[oc][0:rk, :], Act.Tanh)
                        scr = wpool.tile([128, H], f32, name=f"scr_{side}_{ci}_{b}",
                                         tag="scr", bufs=2)
                        tcol = spool.tile([128, 1], f32, name=f"tc_{side}_{ci}_{b}", tag="tcol")
                        nc.vector.scalar_tensor_tensor(
                            out=scr[0:rk, :], in0=th[0:rk, :], scalar=1.0,
                            in1=wr[0:rk, :], op0=Alu.mult, op1=Alu.mult,
                            accum_out=tcol[0:rk, :])
                        acol = spool.tile([128, 1], f16, name=f"a_{side}_{ci}_{b}",
                                          tag=f"acol_{side}_{ci}")
                        nc.scalar.activation(acol[0:rk, :], tcol[0:rk, :], Act.Exp)
                        acols.append((acol, rk))
                        nc.tensor.matmul(
                            s_ps[0:1, 0:1], lhsT=acol[0:rk, 0:1], rhs=ones_col[0:rk, 0:1],
                            start=(ci == 0), stop=(ci == len(rcs) - 1))

                    # -- 1/sum, broadcast to 128 partitions --
                    r_sb = spool.tile([1, 1], f32, name=f"r_{side}_{b}", tag="r")
                    nc.vector.reciprocal(r_sb[0:1, 0:1], s_ps[0:1, 0:1])
                    rb_ps = pstat.tile([128, 1], f32, name=f"rb_{side}_{b}", tag="stat")
                    nc.tensor.matmul(rb_ps[:, 0:1], lhsT=ones_row[0:1, :],
                                     rhs=r_sb[0:1, 0:1])
                    rb_sb = spool.tile([128, 1], f32, name=f"rbs_{side}_{b}", tag="rb")
                    nc.scalar.copy(rb_sb[:, 0:1], rb_ps[:, 0:1])

                    # -- stage 2: out[s, h] = sum_r a_r x[r, h], all 128 s at once --
                    att_ps = [ppool.tile([128, OC], f32, name=f"att{h2}_{side}_{b}",
                                         tag="pp") for h2 in range(2)]
                    for h2 in range(2):
                        for ci, (r0, rk) in enumerate(rcs):
                            acol, _ = acols[ci]
                            nc.tensor.matmul(
                                att_ps[h2][:, :],
                                lhsT=acol[0:rk, 0:1].to_broadcast((rk, 128)),
                                rhs=xn_ts[ci][0:rk, h2 * OC:(h2 + 1) * OC],
                                start=(ci == 0), stop=(ci == len(rcs) - 1))
                    att_sb = opool.tile([128, H], f32, name=f"attsb_{side}_{b}",
                                        tag=f"att_{side}")
                    for h2 in range(2):
                        nc.scalar.activation(att_sb[:, h2 * OC:(h2 + 1) * OC],
                                             att_ps[h2][:, :], Act.Copy,
                                             scale=rb_sb[:, 0:1])
                    for sc in range(S // 128):
                        nc.sync.dma_start(out=out_d[b, sc * 128:(sc + 1) * 128, :],
                                          in_=att_sb[:, :])
    nc.compile()
    return nc


def _get_nc():
    if "nc" not in _CACHE:
        _CACHE["nc"] = build_nc()
    return _CACHE["nc"]


def make_in_maps(inputs):
    dns = np.ascontiguousarray(np.asarray(inputs["dns_feature"], dtype=np.float32))
    img = np.ascontiguousarray(np.asarray(inputs["img_features"], dtype=np.float32))
    W_i1 = np.asarray(inputs["W_i1"], dtype=np.float32)
    W_d2 = np.asarray(inputs["W_d2"], dtype=np.float32)
    wB = np.asarray(inputs["w_att1"], dtype=np.float32)[H:]
    wD = np.asarray(inputs["w_att2"], dtype=np.float32)[H:]

    wt_i1 = np.ascontiguousarray(W_i1.T).reshape(HC, 128, H).astype(np.float16)
    wt_d2 = np.ascontiguousarray(W_d2.T).reshape(HC, 128, H).astype(np.float16)
    wrow_b = np.ascontiguousarray(np.broadcast_to(wB, (128, H)))
    wrow_d = np.ascontiguousarray(np.broadcast_to(wD, (128, H)))

    xt_dns = np.ascontiguousarray(
        dns.transpose(0, 2, 1).reshape(B, HC, 128, S).astype(np.float16))
    xt_img = np.ascontiguousarray(
        img.transpose(0, 2, 1).reshape(B, HC, 128, R).astype(np.float16))
    xn_dns = dns.astype(np.float16)
    xn_img = img.astype(np.float16)

    in_maps = []
    for k in range(NCORES):
        sl = slice(k * BLOC, (k + 1) * BLOC)
        in_maps.append({
            "xt_dns": np.ascontiguousarray(xt_dns[sl]),
            "xn_dns": np.ascontiguousarray(xn_dns[sl]),
            "xt_img": np.ascontiguousarray(xt_img[sl]),
            "xn_img": np.ascontiguousarray(xn_img[sl]),
            "wt_i1": wt_i1,
            "wt_d2": wt_d2,
            "wrow_b": wrow_b,
            "wrow_d": wrow_d,
        })
    return in_maps


def kernel(**inputs):
    from concourse.bass_utils import run_bass_kernel_spmd

    nc = _get_nc()
    in_maps = make_in_maps(inputs)
    res = run_bass_kernel_spmd(nc, in_maps, list(range(NCORES))).results
    att_dns = np.concatenate([res[k]["out_dns"] for k in range(NCORES)], axis=0)
    att_img = np.concatenate([res[k]["out_img"] for k in range(NCORES)], axis=0)
    return att_dns, att_img


# revision 6
# speedup vs baseline: 1.2096x; 1.2096x over previous
"""CoAttention ImageDNS kernel for Trainium2 (8 NeuronCores, Bass/Tile).

Math: the reference computes two additive-attention blocks. In both, the
softmax'd score is  score[b, q, k] = f(q-side)[b, q] + g(k-side)[b, k] + c,
and softmax over k is invariant to the q-dependent (and constant) terms, so
the attention weights are independent of the query index:

  visual_att[b, s, :]  = softmax_r( wB . tanh(W_i1 @ img[b, r]) )
  textual_att[b, i, :] = softmax_j( wD . tanh(W_d2 @ dns[b, j]) )

Hence both outputs are per-batch rank-1 broadcasts:

  att_img_features[b, s, :] = visual_att[b]  @ img[b]   (same for all s)
  att_dns_features[b, i, :] = textual_att[b] @ dns[b]   (same for all i)

W_d1/b_d1/w_att1[:H]/b_att1/W_i2/b_i2/w_att2[:H]/b_att2 cancel entirely.

Sharding: pure data-parallel over batch, 4 batches per core, no collectives.
Matmul operands are bf16 (fp16 streams at half rate on the trn2 PE; bf16
end-to-end rel err ~3e-3 vs the fp32 reference); accumulation is fp32 in
PSUM, softmax/normalization in fp32.
"""

import sys
import numpy as np
import ml_dtypes

_BF16 = ml_dtypes.bfloat16

for _p in ("/opt/trn_rl_repo", "/root/.axon_site/_ro/trn_rl_repo"):
    if _p not in sys.path:
        sys.path.append(_p)

B, S, R, H = 32, 512, 196, 1024
NCORES = 8
BLOC = B // NCORES          # batches per core
OC = 512                    # output-chunk (one fp32 PSUM bank)
HC = H // 128               # contraction chunks

_CACHE = {}


def _row_chunks(n):
    out, o = [], 0
    while o < n:
        out.append((o, min(128, n - o)))
        o += 128
    return out


def build_nc():
    from concourse import bacc, mybir
    from concourse import tile

    f32, f16 = mybir.dt.float32, mybir.dt.bfloat16
    Act = mybir.ActivationFunctionType
    Alu = mybir.AluOpType

    nc = bacc.Bacc("TRN2", target_bir_lowering=False, debug=False)

    xt_dns = nc.dram_tensor("xt_dns", [BLOC, HC, 128, S], f16, kind="ExternalInput")
    xn_dns = nc.dram_tensor("xn_dns", [BLOC, S, H], f16, kind="ExternalInput")
    xt_img = nc.dram_tensor("xt_img", [BLOC, HC, 128, R], f16, kind="ExternalInput")
    xn_img = nc.dram_tensor("xn_img", [BLOC, R, H], f16, kind="ExternalInput")
    wt_i1 = nc.dram_tensor("wt_i1", [HC, 128, H], f16, kind="ExternalInput")
    wt_d2 = nc.dram_tensor("wt_d2", [HC, 128, H], f16, kind="ExternalInput")
    wrow_b = nc.dram_tensor("wrow_b", [128, H], f32, kind="ExternalInput")
    wrow_d = nc.dram_tensor("wrow_d", [128, H], f32, kind="ExternalInput")
    out_dns = nc.dram_tensor("out_dns", [BLOC, S, H], f32, kind="ExternalOutput")
    out_img = nc.dram_tensor("out_img", [BLOC, S, H], f32, kind="ExternalOutput")

    with tile.TileContext(nc) as tc:
        with (
            tc.tile_pool(name="const", bufs=1) as cpool,
            tc.tile_pool(name="xts", bufs=2) as xtpool,
            tc.tile_pool(name="xns", bufs=2) as xnpool,
            tc.tile_pool(name="work", bufs=3) as wpool,
            tc.tile_pool(name="small", bufs=12) as spool,
            tc.tile_pool(name="outs", bufs=2) as opool,
            tc.tile_pool(name="pp", bufs=4, space="PSUM") as ppool,
            tc.tile_pool(name="ps", bufs=3, space="PSUM") as pstat,
        ):
            # lazy const loads: issue each weight's DMAs at first use so the
            # first projection isn't queued behind the other side's weights
            wt_sb, wrow_sb = {}, {}

            def get_wt(nm):
                if nm not in wt_sb:
                    dram = {"i1": wt_i1, "d2": wt_d2}[nm]
                    w = cpool.tile([128, HC * H], f16, name=f"wt_{nm}_sb")
                    for hc in range(HC):
                        nc.sync.dma_start(out=w[:, hc * H:(hc + 1) * H], in_=dram[hc])
                    wt_sb[nm] = w
                return wt_sb[nm]

            def get_wrow(nm):
                if nm not in wrow_sb:
                    dram = {"b": wrow_b, "d": wrow_d}[nm]
                    w = cpool.tile([128, H], f32, name=f"wrow_{nm}_sb")
                    nc.sync.dma_start(out=w[:, :], in_=dram[:, :])
                    wrow_sb[nm] = w
                return wrow_sb[nm]

            ones_col = cpool.tile([128, 1], f16, name="ones_col")
            nc.vector.memset(ones_col[:, :], 1.0)
            ones_row = cpool.tile([1, 128], f32, name="ones_row")
            nc.vector.memset(ones_row[:, :], 1.0)

            for b in range(BLOC):
                for side in ("img", "dns"):
                    n_rows = R if side == "img" else S
                    xt_d = xt_img if side == "img" else xt_dns
                    xn_d = xn_img if side == "img" else xn_dns
                    wt = get_wt("i1" if side == "img" else "d2")
                    wr = get_wrow("b" if side == "img" else "d")
                    out_d = out_img if side == "img" else out_dns
                    rcs = _row_chunks(n_rows)

                    # -- loads --
                    xt_t = xtpool.tile([128, HC * n_rows], f16,
                                       name=f"xt_{side}_{b}", tag=f"xt_{side}")
                    for hc in range(HC):
                        nc.sync.dma_start(
                            out=xt_t[:, hc * n_rows:(hc + 1) * n_rows],
                            in_=xt_d[b, hc])
                    xn_ts = []
                    for ci, (r0, rk) in enumerate(rcs):
                        t = xnpool.tile([128, H], f16, name=f"xn_{side}_{ci}_{b}",
                                        tag=f"xn_{side}_{ci}")
                        nc.sync.dma_start(out=t[0:rk, :], in_=xn_d[b, r0:r0 + rk, :])
                        xn_ts.append(t)

                    # -- projection, tanh, weighted o-reduction, exp --
                    acols = []
                    s_ps = pstat.tile([1, 1], f32, name=f"s_{side}_{b}", tag="stat")
                    for ci, (r0, rk) in enumerate(rcs):
                        ps = [ppool.tile([128, OC], f32, name=f"proj{oc}_{side}_{ci}_{b}",
                                         tag="pp") for oc in range(2)]
                        for hc in range(HC):
                            lhs = xt_t[:, hc * n_rows + r0: hc * n_rows + r0 + rk]
                            for oc in range(2):
                                nc.tensor.matmul(
                                    ps[oc][0:rk, :],
                                    lhsT=lhs,
                                    rhs=wt[:, hc * H + oc * OC: hc * H + (oc + 1) * OC],
                                    start=(hc == 0), stop=(hc == HC - 1))
                        th = wpool.tile([128, H], f32, name=f"th_{side}_{ci}_{b}", tag="th")
                        for oc in range(2):
                            nc.scalar.activation(th[0:rk, oc * OC:(oc + 1) * OC],
                                                 ps[oc][0:rk, :], Act.Tanh)
                        scr = wpool.tile([128, H], f32, name=f"scr_{side}_{ci}_{b}",
                                         tag="scr", bufs=2)
                        tcol = spool.tile([128, 1], f32, name=f"tc_{side}_{ci}_{b}", tag="tcol")
                        nc.vector.scalar_tensor_tensor(
                            out=scr[0:rk, :], in0=th[0:rk, :], scalar=1.0,
                            in1=wr[0:rk, :], op0=Alu.mult, op1=Alu.mult,
                            accum_out=tcol[0:rk, :])
                        acol = spool.tile([128, 1], f16, name=f"a_{side}_{ci}_{b}",
                                          tag=f"acol_{side}_{ci}")
                        nc.scalar.activation(acol[0:rk, :], tcol[0:rk, :], Act.Exp)
                        acols.append((acol, rk))
                        nc.tensor.matmul(
                            s_ps[0:1, 0:1], lhsT=acol[0:rk, 0:1], rhs=ones_col[0:rk, 0:1],
                            start=(ci == 0), stop=(ci == len(rcs) - 1))

                    # -- 1/sum, broadcast to 128 partitions --
                    r_sb = spool.tile([1, 1], f32, name=f"r_{side}_{b}", tag="r")
                    nc.vector.reciprocal(r_sb[0:1, 0:1], s_ps[0:1, 0:1])
                    rb_ps = pstat.tile([128, 1], f32, name=f"rb_{side}_{b}", tag="stat")
                    nc.tensor.matmul(rb_ps[:, 0:1], lhsT=ones_row[0:1, :],
                                     rhs=r_sb[0:1, 0:1])
                    rb_sb = spool.tile([128, 1], f32, name=f"rbs_{side}_{b}", tag="rb")
                    nc.scalar.copy(rb_sb[:, 0:1], rb_ps[:, 0:1])

                    # -- stage 2: out[s, h] = sum_r a_r x[r, h], all 128 s at once --
                    att_ps = [ppool.tile([128, OC], f32, name=f"att{h2}_{side}_{b}",
                                         tag="pp") for h2 in range(2)]
                    for h2 in range(2):
                        for ci, (r0, rk) in enumerate(rcs):
                            acol, _ = acols[ci]
                            nc.tensor.matmul(
                                att_ps[h2][:, :],
                                lhsT=acol[0:rk, 0:1].to_broadcast((rk, 128)),
                                rhs=xn_ts[ci][0:rk, h2 * OC:(h2 + 1) * OC],
                                start=(ci == 0), stop=(ci == len(rcs) - 1))
                    att_sb = opool.tile([128, H], f32, name=f"attsb_{side}_{b}",
                                        tag=f"att_{side}")
                    for h2 in range(2):
                        nc.scalar.activation(att_sb[:, h2 * OC:(h2 + 1) * OC],
                                             att_ps[h2][:, :], Act.Copy,
                                             scale=rb_sb[:, 0:1])
                    for sc in range(S // 128):
                        nc.sync.dma_start(out=out_d[b, sc * 128:(sc + 1) * 128, :],
                                          in_=att_sb[:, :])
    nc.compile()
    return nc


def _get_nc():
    if "nc" not in _CACHE:
        _CACHE["nc"] = build_nc()
    return _CACHE["nc"]


def make_in_maps(inputs):
    dns = np.ascontiguousarray(np.asarray(inputs["dns_feature"], dtype=np.float32))
    img = np.ascontiguousarray(np.asarray(inputs["img_features"], dtype=np.float32))
    W_i1 = np.asarray(inputs["W_i1"], dtype=np.float32)
    W_d2 = np.asarray(inputs["W_d2"], dtype=np.float32)
    wB = np.asarray(inputs["w_att1"], dtype=np.float32)[H:]
    wD = np.asarray(inputs["w_att2"], dtype=np.float32)[H:]

    wt_i1 = np.ascontiguousarray(W_i1.T).reshape(HC, 128, H).astype(_BF16)
    wt_d2 = np.ascontiguousarray(W_d2.T).reshape(HC, 128, H).astype(_BF16)
    wrow_b = np.ascontiguousarray(np.broadcast_to(wB, (128, H)))
    wrow_d = np.ascontiguousarray(np.broadcast_to(wD, (128, H)))

    xt_dns = np.ascontiguousarray(
        dns.transpose(0, 2, 1).reshape(B, HC, 128, S).astype(_BF16))
    xt_img = np.ascontiguousarray(
        img.transpose(0, 2, 1).reshape(B, HC, 128, R).astype(_BF16))
    xn_dns = dns.astype(_BF16)
    xn_img = img.astype(_BF16)

    in_maps = []
    for k in range(NCORES):
        sl = slice(k * BLOC, (k + 1) * BLOC)
        in_maps.append({
            "xt_dns": np.ascontiguousarray(xt_dns[sl]),
            "xn_dns": np.ascontiguousarray(xn_dns[sl]),
            "xt_img": np.ascontiguousarray(xt_img[sl]),
            "xn_img": np.ascontiguousarray(xn_img[sl]),
            "wt_i1": wt_i1,
            "wt_d2": wt_d2,
            "wrow_b": wrow_b,
            "wrow_d": wrow_d,
        })
    return in_maps


def kernel(**inputs):
    from concourse.bass_utils import run_bass_kernel_spmd

    nc = _get_nc()
    in_maps = make_in_maps(inputs)
    res = run_bass_kernel_spmd(nc, in_maps, list(range(NCORES))).results
    att_dns = np.concatenate([res[k]["out_dns"] for k in range(NCORES)], axis=0)
    att_img = np.concatenate([res[k]["out_img"] for k in range(NCORES)], axis=0)
    return att_dns, att_img
